# revision 1
# baseline (speedup 1.0000x reference)
"""Trainium2 Bass kernel for nn_DiffAttn (differential attention).

Reference computation (per batch b):
    Q = X @ Wq.T + bq ; K = X @ Wk.T + bk ; V = X @ Wv.T + bv
    Q1,Q2 / K1,K2 = halves of feature dim
    A_j = (Q_j @ K_j.T) / sqrt(DIM)
    out = softmax(A1) @ V - scalar * softmax(A2) @ V

Sharding: 8 cores = 4 batches x 2 query-halves. Each core computes the
full K/V projection for its batch (redundant within the pair) and the
attention output for its 1024 queries. No collectives needed; output
slabs are disjoint.

Device-side layouts avoid all on-chip transposes: the host pre-transposes
X^T and W^T so every matmul contraction dim lands on SBUF partitions.
Projection / score matmuls run in bf16; P=exp(scores) and V stay fp32
and the attention@V matmuls run as float32r (single-pass fp32, ~2
cycles/column). The attention weights are normalized BEFORE the V matmul
(A = P1/r1 - scalar*P2/r2) so only one attn@V GEMM is needed; row sums
come from an all-ones stationary matmul whose output is replicated
across partitions, and 1/r is computed as exp(-ln r) on the Scalar
engine. Measured on trn2: ~344 us HW exec, rel-err ~2.1e-3 vs the fp32
reference.
"""

import json
import math
import os
from contextlib import ExitStack

import numpy as np
import ml_dtypes

import concourse.bass as bass
import concourse.tile as tile
from concourse import mybir
from concourse.bass_utils import run_bass_kernel_spmd


def _split_waits(raw: bytes, max_waits: int = 1) -> bytes:
    """walrus's CoreV3 codegen rejects instructions carrying more than one
    sync wait ("Too many sync wait commands"); Tile's kernel-tail drain
    aggregates one wait per live processor. Hoist excess waits onto chained
    same-engine Drain instructions inserted immediately before the offender."""
    m = json.loads(raw)
    uid = 0
    for fn in m["functions"]:
        for blk in fn["blocks"]:
            out = []
            for ins in blk["instructions"]:
                sy = ins.get("sync_info") or {}
                waits = sy.get("on_wait") or []
                if len(waits) > max_waits:
                    head, keep = waits[:-max_waits], waits[-max_waits:]
                    while head:
                        chunk, head = head[:max_waits], head[max_waits:]
                        uid += 1
                        out.append(
                            {
                                "engine": ins["engine"],
                                "ins": [],
                                "is_reset_sema": False,
                                "name": f"{ins['name']}-wsplit{uid}",
                                "opcode": "Drain",
                                "outs": [],
                                "sync_info": {"on_update": [], "on_wait": chunk},
                            }
                        )
                    sy["on_wait"] = keep
                out.append(ins)
            blk["instructions"] = out
    return json.dumps(m).encode()

B, S, DIM = 4, 2048, 1024
H = DIM // 2
NCORES = 8
QLEN = S // 2          # queries per core
SCALE = 1.0 / math.sqrt(DIM)

BF16 = mybir.dt.bfloat16
F32 = mybir.dt.float32
F32R = mybir.dt.float32r

DT = DIM // 128        # 8  contraction tiles over model dim
CT = DIM // 128        # 8  feature tiles of Q^T/K^T
KT = S // 128          # 16 key tiles
NQC = QLEN // 512      # 2  query chunks of 512
VW = DIM              # V width (row sums come from an ones-row matmul instead)

# test harness hooks (the grader never touches these)
TRACE = False
LAST_RESULTS = None


def _build_bass():
    nc = bass.Bass(
        trn_type="TRN2",
        target_bir_lowering=False,
        debug=False,
        num_devices=NCORES,
    )

    xt = nc.dram_tensor("xt", [DIM, S], BF16, kind="ExternalInput")
    xtq = nc.dram_tensor("xtq", [DIM, QLEN], BF16, kind="ExternalInput")
    wqt = nc.dram_tensor("wqt", [DIM, DIM], BF16, kind="ExternalInput")
    wkt = nc.dram_tensor("wkt", [DIM, DIM], BF16, kind="ExternalInput")
    wvt = nc.dram_tensor("wvt", [DIM, DIM], BF16, kind="ExternalInput")
    bqr = nc.dram_tensor("bqr", [128, CT], F32, kind="ExternalInput")
    bkr = nc.dram_tensor("bkr", [128, CT], F32, kind="ExternalInput")
    bvb = nc.dram_tensor("bvb", [128, DIM], F32, kind="ExternalInput")
    scv = nc.dram_tensor("scv", [128, 1], F32, kind="ExternalInput")
    outp = nc.dram_tensor("out", [QLEN, DIM], F32, kind="ExternalOutput")

    Id = mybir.ActivationFunctionType.Identity
    Exp = mybir.ActivationFunctionType.Exp
    mult = mybir.AluOpType.mult
    subtract = mybir.AluOpType.subtract

    with tile.TileContext(nc) as tc, ExitStack() as ctx:
        const = ctx.enter_context(tc.tile_pool(name="const", bufs=1))
        persist = ctx.enter_context(tc.tile_pool(name="persist", bufs=1))
        ps_s = ctx.enter_context(
            tc.tile_pool(name="ps_s", bufs=3, space="PSUM")
        )

        bq_sb = const.tile([128, CT], F32)
        nc.sync.dma_start(out=bq_sb[:, :], in_=bqr[:, :])
        bk_sb = const.tile([128, CT], F32)
        nc.sync.dma_start(out=bk_sb[:, :], in_=bkr[:, :])
        sc_sb = const.tile([128, 1], F32)
        nc.sync.dma_start(out=sc_sb[:, :], in_=scv[:, :])
        ones_sb = const.tile([128, 2], F32)
        nc.vector.memset(ones_sb[:, :], 1.0)

        # Warm the PE clock gate (HAM) during the initial input-DMA wait:
        # a chain of tiny dependent matmuls gives ~4.5 us of sustained PE
        # activity so the first projection matmuls run at 2.4 GHz, not 1.2.
        with tc.psum_pool(name="ps_w", bufs=1) as ps_w:
            warm = ps_w.tile([2, 2], F32, name="warm")
            for _ in range(24):
                nc.tensor.matmul(
                    warm[:, :], ones_sb[:, :], ones_sb[:, :], start=True, stop=True
                )

        # persistent products of the projection phase
        q_sb = [persist.tile([128, QLEN], BF16, name=f"q{i}") for i in range(CT)]
        k_sb = [persist.tile([128, S], BF16, name=f"k{i}") for i in range(CT)]
        v_sb = [persist.tile([128, VW], F32R, name=f"v{i}") for i in range(KT)]

        # XT tiles live from before phase 1a through phase 1c (released below)
        xtp = tc.alloc_tile_pool(name="xtp", bufs=1)
        x_t = [xtp.tile([128, S], BF16, name=f"x{d}") for d in range(DT)]

        # wk prefetch pool outlives phase 1a (released after phase 1c)
        wkpre = tc.alloc_tile_pool(name="wkpre", bufs=1)
        wk_pre = [wkpre.tile([128, DIM], BF16, name=f"wkp{d}") for d in range(4)]

        # ---- Phase 1a: Q^T[c, q] = Wq^T.T @ X^T[:, qsel]  (+bq) ----
        with nc.named_scope("proj_q"), tc.tile_pool(name="wq", bufs=1) as wqp, tc.tile_pool(
            name="xq", bufs=1
        ) as xqp:
            wq_t = [wqp.tile([128, DIM], BF16, name=f"wq{d}") for d in range(DT)]
            xq_t = [xqp.tile([128, QLEN], BF16, name=f"xq{d}") for d in range(DT)]
            for d in range(DT):
                nc.sync.dma_start(out=xq_t[d][:, :], in_=xtq[d * 128 : (d + 1) * 128, :])
                nc.sync.dma_start(out=wq_t[d][:, :], in_=wqt[d * 128 : (d + 1) * 128, :])
            for d in range(DT):
                nc.sync.dma_start(out=x_t[d][:, :], in_=xt[d * 128 : (d + 1) * 128, :])
            for d in range(4):
                nc.sync.dma_start(out=wk_pre[d][:, :], in_=wkt[d * 128 : (d + 1) * 128, :])
            for c in range(CT):
                for n in range(QLEN // 512):
                    ps = ps_s.tile([128, 512], F32, tag="ps", name="psq")
                    for d in range(DT):
                        nc.tensor.matmul(
                            ps[:, :],
                            wq_t[d][:, c * 128 : (c + 1) * 128],
                            xq_t[d][:, n * 512 : (n + 1) * 512],
                            start=(d == 0),
                            stop=(d == DT - 1),
                        )
                    nc.scalar.activation(
                        q_sb[c][:, n * 512 : (n + 1) * 512],
                        ps[:, :],
                        Id,
                        bias=bq_sb[:, c : c + 1],
                    )

        # ---- Phase 1b: K^T[c, k] = Wk^T.T @ X^T  (+bk) ----
        with nc.named_scope("proj_kv"), tc.tile_pool(name="wk", bufs=1) as wkp:
            wk_t = wk_pre + [
                wkp.tile([128, DIM], BF16, name=f"wk{d}") for d in range(4, DT)
            ]
            for d in range(4, DT):
                nc.sync.dma_start(out=wk_t[d][:, :], in_=wkt[d * 128 : (d + 1) * 128, :])
            for c in range(CT):
                for n in range(S // 512):
                    ps = ps_s.tile([128, 512], F32, tag="ps", name="psk")
                    for d in range(DT):
                        nc.tensor.matmul(
                            ps[:, :],
                            wk_t[d][:, c * 128 : (c + 1) * 128],
                            x_t[d][:, n * 512 : (n + 1) * 512],
                            start=(d == 0),
                            stop=(d == DT - 1),
                        )
                    nc.scalar.activation(
                        k_sb[c][:, n * 512 : (n + 1) * 512],
                        ps[:, :],
                        Id,
                        bias=bk_sb[:, c : c + 1],
                    )

            # ---- Phase 1c: V[k, d] = X^T.T @ Wv^T  (+bv broadcast) ----
            # x_t (X^T tiles) stay resident as the stationary operand.
            with tc.tile_pool(name="wv", bufs=1) as wvp:
                bv_sb = wvp.tile([128, DIM], F32, name="bv_sb")
                nc.sync.dma_start(out=bv_sb[:, :], in_=bvb[:, :])
                wv_t = [wvp.tile([128, DIM], BF16, name=f"wv{d}") for d in range(DT)]
                for d in range(DT):
                    nc.sync.dma_start(
                        out=wv_t[d][:, :], in_=wvt[d * 128 : (d + 1) * 128, :]
                    )
                for k in range(KT):
                    for n in range(DIM // 512):
                        ps = ps_s.tile([128, 512], F32, tag="ps", name="psv")
                        for d in range(DT):
                            nc.tensor.matmul(
                                ps[:, :],
                                x_t[d][:, k * 128 : (k + 1) * 128],
                                wv_t[d][:, n * 512 : (n + 1) * 512],
                                start=(d == 0),
                                stop=(d == DT - 1),
                            )
                        nc.vector.tensor_add(
                            v_sb[k][:, n * 512 : (n + 1) * 512],
                            ps[:, :],
                            bv_sb[:, n * 512 : (n + 1) * 512],
                        )

        wkpre.release()
        xtp.release()

        # ---- Phase 2: attention, one 512-query chunk at a time ----
        # Normalize P before the V matmul so only ONE attn@V GEMM is needed:
        #   A^T = P1^T * bcast(1/r1) - P2^T * bcast(scalar/r2);  out = A^T.T @ V
        # r_j comes from an ones-row stationary matmul (column sums of P^T);
        # bcast replicates the [1, q] reciprocal row across partitions via a
        # K=1 ones-column matmul.
        lnsc_sb = const.tile([128, 1], F32)
        nc.scalar.activation(lnsc_sb[:, :], sc_sb[:, :], mybir.ActivationFunctionType.Ln)
        ones_sq = const.tile([128, 128], F32R)
        ones_sqf = const.tile([128, 128], F32)
        nc.vector.memset(ones_sqf[:, :], 1.0)
        nc.vector.tensor_copy(ones_sq[:, :], ones_sqf[:, :])

        with (
            tc.tile_pool(name="pP", bufs=1) as pP,
            tc.tile_pool(name="ps_r", bufs=1, space="PSUM") as ps_r,
            tc.tile_pool(name="ps_u", bufs=4, space="PSUM") as ps_u,
            tc.tile_pool(name="small", bufs=4) as small,
            tc.tile_pool(name="tmp2", bufs=2) as tmp2,
            tc.tile_pool(name="ostage", bufs=2) as ostage,
        ):
            p_sb = [
                [pP.tile([128, 512], F32R, name=f"p{j}_{k}") for k in range(KT)]
                for j in range(2)
            ]
            for qc in range(NQC):
                # scores S^T[k, q] = K_j^T.T @ Q_j^T; P = exp(s*S^T); r = col sums
                bcs = []
                scope_s = nc.enter_named_scope(f"attn_s{qc}", False)
                for j in range(2):
                    # r replicated across partitions: ones[128,128].T @ P = col sums
                    r_ps = ps_r.tile([128, 512], F32, tag="r", name=f"r{j}")
                    for k in range(KT):
                        ps = ps_s.tile([128, 512], F32, tag="ps", name="pss")
                        for ci in range(4):
                            c = 4 * j + ci
                            nc.tensor.matmul(
                                ps[:, :],
                                k_sb[c][:, k * 128 : (k + 1) * 128],
                                q_sb[c][:, qc * 512 : (qc + 1) * 512],
                                start=(ci == 0),
                                stop=(ci == 3),
                            )
                        nc.scalar.activation(
                            p_sb[j][k][:, :], ps[:, :], Exp, scale=SCALE
                        )
                        nc.tensor.matmul(
                            r_ps[:, :],
                            ones_sq[:, :],
                            p_sb[j][k][:, :],
                            start=(k == 0),
                            stop=(k == KT - 1),
                        )
                    # bc_j = exp(-ln r_j) = 1/r_j on the Scalar engine
                    # (j=1 folds the input scalar in via a +ln(scalar) bias)
                    lnr = tmp2.tile([128, 512], F32, tag="lnr", name="lnr")
                    nc.scalar.activation(
                        lnr[:, :], r_ps[:, :], mybir.ActivationFunctionType.Ln
                    )
                    bc = small.tile([128, 512], F32, tag=f"bc{j}", name=f"bc{j}")
                    if j == 0:
                        nc.scalar.activation(bc[:, :], lnr[:, :], Exp, scale=-1.0)
                    else:
                        nc.scalar.activation(
                            bc[:, :], lnr[:, :], Exp, scale=-1.0, bias=lnsc_sb[:, :]
                        )
                    bcs.append(bc)
                nc.leave_named_scope(f"attn_s{qc}", scope_s[0], False)

                # A^T[k] = P1[k]*bc1 - P2[k]*bc2s  (in place into p_sb[0])
                scope_a = nc.enter_named_scope(f"attn_a{qc}", False)
                for k in range(KT):
                    t2 = tmp2.tile([128, 512], F32, tag="t2", name="t2")
                    nc.vector.tensor_mul(t2[:, :], p_sb[0][k][:, :], bcs[0][:, :])
                    nc.vector.tensor_mul(
                        p_sb[1][k][:, :], p_sb[1][k][:, :], bcs[1][:, :]
                    )
                    nc.vector.tensor_sub(p_sb[1][k][:, :], t2[:, :], p_sb[1][k][:, :])
                nc.leave_named_scope(f"attn_a{qc}", scope_a[0], False)

                # out rows = A^T.T @ V
                scope_u = nc.enter_named_scope(f"attn_u{qc}", False)
                for t in range(4):
                    row = qc * 512 + t * 128
                    for n in range(DIM // 512):
                        lo, hi = n * 512, (n + 1) * 512
                        u = ps_u.tile([128, 512], F32, tag="u", name="u")
                        for k in range(KT):
                            nc.tensor.matmul(
                                u[:, :],
                                p_sb[1][k][:, t * 128 : (t + 1) * 128],
                                v_sb[k][:, lo:hi],
                                start=(k == 0),
                                stop=(k == KT - 1),
                            )
                        o = ostage.tile([128, 512], F32, tag="o", name="o")
                        if n == 0:
                            nc.scalar.copy(o[:, :], u[:, :])
                        else:
                            nc.vector.tensor_copy(o[:, :], u[:, :])
                        nc.sync.dma_start(
                            out=outp[row : row + 128, lo:hi], in_=o[:, :]
                        )
                nc.leave_named_scope(f"attn_u{qc}", scope_u[0], False)

    return nc


_NC_CACHE = None


def _get_nc():
    global _NC_CACHE
    if _NC_CACHE is None:
        nc = _build_bass()
        fixed = _split_waits(bass.Bass.to_json_bytes(nc))
        nc.to_json_bytes = lambda: fixed
        _NC_CACHE = nc
    return _NC_CACHE


def kernel(hidden_states, W_q, b_q, W_k, b_k, W_v, b_v, scalar):
    global LAST_RESULTS
    bf16 = ml_dtypes.bfloat16
    X = np.asarray(hidden_states, np.float32)
    wqt = np.ascontiguousarray(np.asarray(W_q, np.float32).T).astype(bf16)
    wkt = np.ascontiguousarray(np.asarray(W_k, np.float32).T).astype(bf16)
    wvt = np.ascontiguousarray(np.asarray(W_v, np.float32).T).astype(bf16)
    bqr = np.ascontiguousarray(np.asarray(b_q, np.float32).reshape(CT, 128).T)
    bkr = np.ascontiguousarray(np.asarray(b_k, np.float32).reshape(CT, 128).T)
    bvb = np.ascontiguousarray(
        np.broadcast_to(np.asarray(b_v, np.float32), (128, DIM))
    )
    scv = np.full((128, 1), np.asarray(scalar, np.float32).reshape(-1)[0], np.float32)

    in_maps = []
    xts = {}
    for core in range(NCORES):
        b, h = core // 2, core % 2
        if b not in xts:
            xts[b] = np.ascontiguousarray(X[b].T).astype(bf16)
        xt_b = xts[b]
        xtq = np.ascontiguousarray(xt_b[:, h * QLEN : (h + 1) * QLEN])
        in_maps.append(
            {
                "xt": xt_b,
                "xtq": xtq,
                "wqt": wqt,
                "wkt": wkt,
                "wvt": wvt,
                "bqr": bqr,
                "bkr": bkr,
                "bvb": bvb,
                "scv": scv,
            }
        )

    nc = _get_nc()
    res = run_bass_kernel_spmd(
        nc,
        in_maps,
        list(range(NCORES)),
        trace=TRACE,
    )
    LAST_RESULTS = res

    out = np.empty((B, S, DIM), np.float32)
    for core in range(NCORES):
        b, h = core // 2, core % 2
        out[b, h * QLEN : (h + 1) * QLEN, :] = res.results[core]["out"]
    return out


if __name__ == "__main__":
    import reference

    inputs = {k: np.asarray(v) for k, v in reference.setup_inputs().items()}
    got = kernel(**inputs)
    print("kernel output", got.shape, got.dtype)



# revision 3
# speedup vs baseline: 1.1918x; 1.1918x over previous
"""Trainium2 Bass kernel for nn_DiffAttn (differential attention).

Reference computation (per batch b):
    Q = X @ Wq.T + bq ; K = X @ Wk.T + bk ; V = X @ Wv.T + bv
    Q1,Q2 / K1,K2 = halves of feature dim
    A_j = (Q_j @ K_j.T) / sqrt(DIM)
    out = softmax(A1) @ V - scalar * softmax(A2) @ V

Sharding: 8 cores = 4 batches x 2 token-halves. Each core projects
Q/K/V only for its OWN 1024 tokens; the K^T/V halves are exchanged
inside each batch pair with two pair-wise AllGather collectives
(HBM bounce buffers), so no projection work is duplicated. The gather
output is rank-ordered == key-half-ordered, so every core addresses
K/V tiles by global key index and the program is SPMD-uniform; the
only per-core data is the X^T token slab (and the host assembles the
output slabs).

All matmuls run in bf16 (fp32r moving operands measure ~2x slower per
column on TRN2 hardware): projections, scores, rowsums (ones-matmul),
and attn@V. P = exp(scores) is stored bf16; attention weights are
normalized BEFORE the V matmul (A = P1/r1 - scalar*P2/r2, with 1/r =
exp(-ln r) on the Scalar engine) so a single attn@V GEMM suffices.
The two query chunks are software-pipelined: the DVE combine of chunk
0 overlaps the score matmuls of chunk 1, and rowsum matmuls trail the
score chains by two tiles so the Scalar-engine exp latency stays off
the PE critical path. Output is written bf16 and widened on the host.
"""

import json
import math
from contextlib import ExitStack

import numpy as np
import ml_dtypes

import concourse.bass as bass
import concourse.tile as tile
from concourse import mybir
from concourse.bass_utils import run_bass_kernel_spmd


def _split_waits(raw: bytes, max_waits: int = 1) -> bytes:
    """walrus's CoreV3 codegen rejects instructions carrying more than one
    sync wait ("Too many sync wait commands"); Tile's kernel-tail drain
    aggregates one wait per live processor. Hoist excess waits onto chained
    same-engine Drain instructions inserted immediately before the offender."""
    m = json.loads(raw)
    uid = 0
    for fn in m["functions"]:
        for blk in fn["blocks"]:
            out = []
            for ins in blk["instructions"]:
                sy = ins.get("sync_info") or {}
                waits = sy.get("on_wait") or []
                if len(waits) > max_waits:
                    head, keep = waits[:-max_waits], waits[-max_waits:]
                    while head:
                        chunk, head = head[:max_waits], head[max_waits:]
                        uid += 1
                        out.append(
                            {
                                "engine": ins["engine"],
                                "ins": [],
                                "is_reset_sema": False,
                                "name": f"{ins['name']}-wsplit{uid}",
                                "opcode": "Drain",
                                "outs": [],
                                "sync_info": {"on_update": [], "on_wait": chunk},
                            }
                        )
                    sy["on_wait"] = keep
                out.append(ins)
            blk["instructions"] = out
    return json.dumps(m).encode()


B, S, DIM = 4, 2048, 1024
H = DIM // 2
NCORES = 8
TOK = S // 2           # tokens (queries and keys) owned per core
SCALE = 1.0 / math.sqrt(DIM)

BF16 = mybir.dt.bfloat16
F32 = mybir.dt.float32

DT = DIM // 128        # 8  contraction tiles over model dim
CT = DIM // 128        # 8  feature tiles of Q^T/K^T
KT = S // 128          # 16 key tiles (global)
KTH = TOK // 128       # 8  key tiles per core half
NQC = TOK // 512       # 2  query chunks of 512

# test harness hooks (the grader never touches these)
TRACE = False
LAST_RESULTS = None


def _build_bass():
    nc = bass.Bass(
        trn_type="TRN2",
        target_bir_lowering=False,
        debug=False,
        num_devices=NCORES,
    )

    xth = nc.dram_tensor("xth", [DIM, TOK], BF16, kind="ExternalInput")
    wqt = nc.dram_tensor("wqt", [DIM, DIM], BF16, kind="ExternalInput")
    wkt = nc.dram_tensor("wkt", [DIM, DIM], BF16, kind="ExternalInput")
    wvt = nc.dram_tensor("wvt", [DIM, DIM], BF16, kind="ExternalInput")
    bqr = nc.dram_tensor("bqr", [128, CT], F32, kind="ExternalInput")
    bkr = nc.dram_tensor("bkr", [128, CT], F32, kind="ExternalInput")
    bvb = nc.dram_tensor("bvb", [128, DIM], F32, kind="ExternalInput")
    scv = nc.dram_tensor("scv", [128, 1], F32, kind="ExternalInput")
    outp = nc.dram_tensor("out", [TOK, DIM], BF16, kind="ExternalOutput")

    Id = mybir.ActivationFunctionType.Identity
    Exp = mybir.ActivationFunctionType.Exp
    Ln = mybir.ActivationFunctionType.Ln
    PAIRS = [[0, 1], [2, 3], [4, 5], [6, 7]]

    with tile.TileContext(nc) as tc, ExitStack() as ctx:
        const = ctx.enter_context(tc.tile_pool(name="const", bufs=1))
        persist = ctx.enter_context(tc.tile_pool(name="persist", bufs=1))
        dram = ctx.enter_context(tc.tile_pool(name="dram", bufs=1, space="DRAM"))
        ps_s = ctx.enter_context(tc.tile_pool(name="ps_s", bufs=3, space="PSUM"))

        bq_sb = const.tile([128, CT], F32)
        nc.sync.dma_start(out=bq_sb[:, :], in_=bqr[:, :])
        bk_sb = const.tile([128, CT], F32)
        nc.sync.dma_start(out=bk_sb[:, :], in_=bkr[:, :])
        sc_sb = const.tile([128, 1], F32)
        nc.sync.dma_start(out=sc_sb[:, :], in_=scv[:, :])
        ones_sb = const.tile([128, 2], F32)
        nc.vector.memset(ones_sb[:, :], 1.0)

        # Warm the PE clock gate (HAM) during the initial input-DMA wait:
        # a chain of tiny dependent matmuls gives ~4.5 us of sustained PE
        # activity so the first projection matmuls run at 2.4 GHz, not 1.2.
        with tc.psum_pool(name="ps_w", bufs=1) as ps_w:
            warm = ps_w.tile([2, 2], F32, name="warm")
            for _ in range(24):
                nc.tensor.matmul(
                    warm[:, :], ones_sb[:, :], ones_sb[:, :], start=True, stop=True
                )

        # ones stationary for the rowsum matmul (column sums, replicated
        # across partitions)
        ones_f = const.tile([128, 128], F32)
        nc.vector.memset(ones_f[:, :], 1.0)
        ones_bf = const.tile([128, 128], BF16)
        nc.vector.tensor_copy(ones_bf[:, :], ones_f[:, :])
        lnsc_sb = const.tile([128, 1], F32)
        nc.scalar.activation(lnsc_sb[:, :], sc_sb[:, :], Ln)

        # persistent products
        q_sb = [persist.tile([128, TOK], BF16, name=f"q{i}") for i in range(CT)]
        k_sb = [persist.tile([128, S], BF16, name=f"k{i}") for i in range(CT)]
        v_sb = [persist.tile([128, DIM], BF16, name=f"v{i}") for i in range(KT)]

        # DRAM bounce buffers for the pair exchange
        k_send = dram.tile([128, CT * TOK], BF16, name="k_send")
        k_recv = dram.tile([2, 128, CT * TOK], BF16, name="k_recv")
        v_send = dram.tile([128, KTH * DIM], BF16, name="v_send")
        v_recv = dram.tile([2, 128, KTH * DIM], BF16, name="v_recv")

        # XT tiles feed K, V and Q projections
        xtp = tc.alloc_tile_pool(name="xtp", bufs=1)
        x_t = [xtp.tile([128, TOK], BF16, name=f"x{d}") for d in range(DT)]

        wvpre = tc.alloc_tile_pool(name="wvpre", bufs=1)
        wv_pre = [wvpre.tile([128, DIM], BF16, name=f"wvp{d}") for d in range(4)]

        # ---- Phase K: K^T[c, own keys] = Wk^T.T @ X^T  (+bk) ----
        kstage = tc.alloc_tile_pool(name="kstage", bufs=1)
        with tc.tile_pool(name="wk", bufs=1) as wkp:
            wk_t = [wkp.tile([128, DIM], BF16, name=f"wk{d}") for d in range(DT)]
            for d in range(DT):
                nc.sync.dma_start(out=x_t[d][:, :], in_=xth[d * 128 : (d + 1) * 128, :])
                nc.sync.dma_start(out=wk_t[d][:, :], in_=wkt[d * 128 : (d + 1) * 128, :])
            for d in range(4):
                nc.sync.dma_start(out=wv_pre[d][:, :], in_=wvt[d * 128 : (d + 1) * 128, :])
            k_st = [kstage.tile([128, TOK], BF16, name=f"ks{c}") for c in range(CT)]
            for c in range(CT):
                for n in range(TOK // 512):
                    ps = ps_s.tile([128, 512], F32, tag="ps", name="psk")
                    for d in range(DT):
                        nc.tensor.matmul(
                            ps[:, :],
                            wk_t[d][:, c * 128 : (c + 1) * 128],
                            x_t[d][:, n * 512 : (n + 1) * 512],
                            start=(d == 0),
                            stop=(d == DT - 1),
                        )
                    nc.scalar.activation(
                        k_st[c][:, n * 512 : (n + 1) * 512],
                        ps[:, :],
                        Id,
                        bias=bk_sb[:, c : c + 1],
                    )
                nc.sync.dma_start(
                    out=k_send[:, c * TOK : (c + 1) * TOK], in_=k_st[c][:, :]
                )

        # pair-wise AllGather #1: K^T halves (rank order == key-half order)
        nc.gpsimd.collective_compute(
            "AllGather",
            mybir.AluOpType.bypass,
            replica_groups=PAIRS,
            ins=[k_send[:, :].opt()],
            outs=[k_recv[:, :, :].opt()],
        )
        for g in range(2):
            for c in range(CT):
                nc.sync.dma_start(
                    out=k_sb[c][:, g * TOK : (g + 1) * TOK],
                    in_=k_recv[g, :, c * TOK : (c + 1) * TOK],
                )

        # ---- Phase V: V[own keys, :] = X^T.T @ Wv^T  (+bv) ----
        vstage = tc.alloc_tile_pool(name="vstage", bufs=1)
        with tc.tile_pool(name="wv", bufs=1) as wvp:
            bv_sb = wvp.tile([128, DIM], F32, name="bv_sb")
            nc.sync.dma_start(out=bv_sb[:, :], in_=bvb[:, :])
            wv_t = wv_pre + [
                wvp.tile([128, DIM], BF16, name=f"wv{d}") for d in range(4, DT)
            ]
            for d in range(4, DT):
                nc.sync.dma_start(out=wv_t[d][:, :], in_=wvt[d * 128 : (d + 1) * 128, :])
            v_st = [vstage.tile([128, DIM], BF16, name=f"vs{i}") for i in range(KTH)]
            for i in range(KTH):
                for n in range(DIM // 512):
                    ps = ps_s.tile([128, 512], F32, tag="ps", name="psv")
                    for d in range(DT):
                        nc.tensor.matmul(
                            ps[:, :],
                            x_t[d][:, i * 128 : (i + 1) * 128],
                            wv_t[d][:, n * 512 : (n + 1) * 512],
                            start=(d == 0),
                            stop=(d == DT - 1),
                        )
                    nc.vector.tensor_add(
                        v_st[i][:, n * 512 : (n + 1) * 512],
                        ps[:, :],
                        bv_sb[:, n * 512 : (n + 1) * 512],
                    )
                nc.sync.dma_start(
                    out=v_send[:, i * DIM : (i + 1) * DIM], in_=v_st[i][:, :]
                )

        # pair-wise AllGather #2: V halves
        nc.gpsimd.collective_compute(
            "AllGather",
            mybir.AluOpType.bypass,
            replica_groups=PAIRS,
            ins=[v_send[:, :].opt()],
            outs=[v_recv[:, :, :].opt()],
        )
        for g in range(2):
            for i in range(KTH):
                nc.sync.dma_start(
                    out=v_sb[g * KTH + i][:, :],
                    in_=v_recv[g, :, i * DIM : (i + 1) * DIM],
                )

        # ---- Phase Q: Q^T[c, own queries] = Wq^T.T @ X^T  (+bq) ----
        with tc.tile_pool(name="wq", bufs=1) as wqp:
            wq_t = [wqp.tile([128, DIM], BF16, name=f"wq{d}") for d in range(DT)]
            for d in range(DT):
                nc.sync.dma_start(out=wq_t[d][:, :], in_=wqt[d * 128 : (d + 1) * 128, :])
            for c in range(CT):
                for n in range(TOK // 512):
                    ps = ps_s.tile([128, 512], F32, tag="ps", name="psq")
                    for d in range(DT):
                        nc.tensor.matmul(
                            ps[:, :],
                            wq_t[d][:, c * 128 : (c + 1) * 128],
                            x_t[d][:, n * 512 : (n + 1) * 512],
                            start=(d == 0),
                            stop=(d == DT - 1),
                        )
                    nc.scalar.activation(
                        q_sb[c][:, n * 512 : (n + 1) * 512],
                        ps[:, :],
                        Id,
                        bias=bq_sb[:, c : c + 1],
                    )
        vstage.release()
        kstage.release()
        wvpre.release()
        xtp.release()

        # ---- Phase 2: attention ----
        # P^T tiles for both query chunks stay live so chunk 1's score
        # matmuls overlap chunk 0's DVE combine.
        with (
            tc.tile_pool(name="pP", bufs=1) as pP,
            tc.tile_pool(name="ps_r", bufs=2, space="PSUM") as ps_r,
            tc.tile_pool(name="ps_u", bufs=3, space="PSUM") as ps_u,
            tc.tile_pool(name="small", bufs=8) as small,
            tc.tile_pool(name="tmp2", bufs=2) as tmp2,
            tc.tile_pool(name="ostage", bufs=3) as ostage,
        ):
            p_sb = [
                [
                    [pP.tile([128, 512], BF16, name=f"p{qc}_{j}_{k}") for k in range(KT)]
                    for j in range(2)
                ]
                for qc in range(NQC)
            ]
            bcs = [[None, None] for _ in range(NQC)]

            def emit_scores(qc):
                # scores S^T[k, q] = K_j^T.T @ Q_j^T; P = exp(s*S^T);
                # r = column sums via ones-matmul, trailed by 2 tiles so the
                # Scalar exp latency stays off the PE critical path.
                for j in range(2):
                    r_ps = ps_r.tile([128, 512], F32, tag="r", name=f"r{qc}{j}")
                    for k in range(KT):
                        ps = ps_s.tile([128, 512], F32, tag="ps", name="pss")
                        for ci in range(4):
                            c = 4 * j + ci
                            nc.tensor.matmul(
                                ps[:, :],
                                k_sb[c][:, k * 128 : (k + 1) * 128],
                                q_sb[c][:, qc * 512 : (qc + 1) * 512],
                                start=(ci == 0),
                                stop=(ci == 3),
                            )
                        nc.scalar.activation(
                            p_sb[qc][j][k][:, :], ps[:, :], Exp, scale=SCALE
                        )
                        if k >= 2:
                            nc.tensor.matmul(
                                r_ps[:, :],
                                ones_bf[:, :],
                                p_sb[qc][j][k - 2][:, :],
                                start=(k == 2),
                                stop=False,
                            )
                    for k in (KT - 2, KT - 1):
                        nc.tensor.matmul(
                            r_ps[:, :],
                            ones_bf[:, :],
                            p_sb[qc][j][k][:, :],
                            start=False,
                            stop=(k == KT - 1),
                        )
                    # bc_j = exp(-ln r_j) = 1/r_j on the Scalar engine
                    # (j=1 folds the input scalar in via a +ln(scalar) bias)
                    lnr = tmp2.tile([128, 512], F32, tag="lnr", name="lnr")
                    nc.scalar.activation(lnr[:, :], r_ps[:, :], Ln)
                    bc = small.tile([128, 512], BF16, tag=f"bc{qc}{j}", name=f"bc{qc}{j}")
                    if j == 0:
                        nc.scalar.activation(bc[:, :], lnr[:, :], Exp, scale=-1.0)
                    else:
                        nc.scalar.activation(
                            bc[:, :], lnr[:, :], Exp, scale=-1.0, bias=lnsc_sb[:, :]
                        )
                    bcs[qc][j] = bc

            def emit_combine(qc):
                # A^T[k] = P1[k]*bc1 - P2[k]*bc2s  (in place into p_sb[qc][1])
                for k in range(KT):
                    t2 = tmp2.tile([128, 512], BF16, tag="t2", name="t2")
                    nc.vector.tensor_mul(t2[:, :], p_sb[qc][0][k][:, :], bcs[qc][0][:, :])
                    nc.vector.tensor_mul(
                        p_sb[qc][1][k][:, :], p_sb[qc][1][k][:, :], bcs[qc][1][:, :]
                    )
                    nc.vector.tensor_sub(
                        p_sb[qc][1][k][:, :], t2[:, :], p_sb[qc][1][k][:, :]
                    )

            def emit_attnv(qc):
                # out rows = A^T.T @ V
                for t in range(4):
                    row = qc * 512 + t * 128
                    for n in range(DIM // 512):
                        lo, hi = n * 512, (n + 1) * 512
                        u = ps_u.tile([128, 512], F32, tag="u", name="u")
                        for k in range(KT):
                            nc.tensor.matmul(
                                u[:, :],
                                p_sb[qc][1][k][:, t * 128 : (t + 1) * 128],
                                v_sb[k][:, lo:hi],
                                start=(k == 0),
                                stop=(k == KT - 1),
                            )
                        o = ostage.tile([128, 512], BF16, tag="o", name="o")
                        if n == 0:
                            nc.scalar.copy(o[:, :], u[:, :])
                        else:
                            nc.vector.tensor_copy(o[:, :], u[:, :])
                        nc.sync.dma_start(out=outp[row : row + 128, lo:hi], in_=o[:, :])

            emit_scores(0)
            emit_combine(0)
            emit_scores(1)
            emit_combine(1)
            emit_attnv(0)
            emit_attnv(1)

    return nc


_NC_CACHE = None


def _get_nc():
    global _NC_CACHE
    if _NC_CACHE is None:
        nc = _build_bass()
        fixed = _split_waits(bass.Bass.to_json_bytes(nc))
        nc.to_json_bytes = lambda: fixed
        _NC_CACHE = nc
    return _NC_CACHE


def kernel(hidden_states, W_q, b_q, W_k, b_k, W_v, b_v, scalar):
    global LAST_RESULTS
    bf16 = ml_dtypes.bfloat16
    X = np.asarray(hidden_states, np.float32)
    wqt = np.ascontiguousarray(np.asarray(W_q, np.float32).T).astype(bf16)
    wkt = np.ascontiguousarray(np.asarray(W_k, np.float32).T).astype(bf16)
    wvt = np.ascontiguousarray(np.asarray(W_v, np.float32).T).astype(bf16)
    bqr = np.ascontiguousarray(np.asarray(b_q, np.float32).reshape(CT, 128).T)
    bkr = np.ascontiguousarray(np.asarray(b_k, np.float32).reshape(CT, 128).T)
    bvb = np.ascontiguousarray(
        np.broadcast_to(np.asarray(b_v, np.float32), (128, DIM))
    )
    scv = np.full((128, 1), np.asarray(scalar, np.float32).reshape(-1)[0], np.float32)

    in_maps = []
    for core in range(NCORES):
        b, h = core // 2, core % 2
        xt_b = np.ascontiguousarray(X[b, h * TOK : (h + 1) * TOK, :].T).astype(bf16)
        in_maps.append(
            {
                "xth": xt_b,
                "wqt": wqt,
                "wkt": wkt,
                "wvt": wvt,
                "bqr": bqr,
                "bkr": bkr,
                "bvb": bvb,
                "scv": scv,
            }
        )

    nc = _get_nc()
    res = run_bass_kernel_spmd(
        nc,
        in_maps,
        list(range(NCORES)),
        trace=TRACE,
    )
    LAST_RESULTS = res

    out = np.empty((B, S, DIM), np.float32)
    for core in range(NCORES):
        b, h = core // 2, core % 2
        out[b, h * TOK : (h + 1) * TOK, :] = res.results[core]["out"].astype(np.float32)
    return out


if __name__ == "__main__":
    import reference

    inputs = {k: np.asarray(v) for k, v in reference.setup_inputs().items()}
    got = kernel(**inputs)
    print("kernel output", got.shape, got.dtype)


# revision 7
# speedup vs baseline: 1.3053x; 1.0953x over previous
"""Trainium2 Bass kernel for nn_DiffAttn (differential attention).

Reference computation (per batch b):
    Q = X @ Wq.T + bq ; K = X @ Wk.T + bk ; V = X @ Wv.T + bv
    Q1,Q2 / K1,K2 = halves of feature dim
    A_j = (Q_j @ K_j.T) / sqrt(DIM)
    out = softmax(A1) @ V - scalar * softmax(A2) @ V

Sharding: 8 cores = 4 batches x 2 token-halves. Each core projects
Q/K/V only for its OWN 1024 tokens; the K^T/V halves are exchanged
inside each batch pair with two pair-wise AllGather collectives
(HBM bounce buffers), so no projection work is duplicated. The gather
output is rank-ordered == key-half-ordered, so every core addresses
K/V tiles by global key index and the program is SPMD-uniform; the
only per-core data is the X^T token slab (and the host assembles the
output slabs).

All matmuls run in bf16 (fp32r moving operands measure ~2x slower per
column on TRN2 hardware): projections, scores, rowsums (ones-matmul),
and attn@V. P = exp(scores) is stored bf16; attention weights are
normalized BEFORE the V matmul (A = P1/r1 - scalar*P2/r2, with 1/r =
exp(-ln r) on the Scalar engine) so a single attn@V GEMM suffices.
The two query chunks are software-pipelined: the DVE combine of chunk
0 overlaps the score matmuls of chunk 1, and rowsum matmuls trail the
score chains by two tiles so the Scalar-engine exp latency stays off
the PE critical path. Output is written bf16 and widened on the host.
"""

import json
import math
from contextlib import ExitStack

import numpy as np
import ml_dtypes

import concourse.bass as bass
import concourse.tile as tile
from concourse import mybir
from concourse.bass_utils import run_bass_kernel_spmd


def _split_waits(raw: bytes, max_waits: int = 1) -> bytes:
    """walrus's CoreV3 codegen rejects instructions carrying more than one
    sync wait ("Too many sync wait commands"); Tile's kernel-tail drain
    aggregates one wait per live processor. Hoist excess waits onto chained
    same-engine Drain instructions inserted immediately before the offender."""
    m = json.loads(raw)
    uid = 0
    for fn in m["functions"]:
        for blk in fn["blocks"]:
            out = []
            for ins in blk["instructions"]:
                sy = ins.get("sync_info") or {}
                waits = sy.get("on_wait") or []
                if len(waits) > max_waits:
                    head, keep = waits[:-max_waits], waits[-max_waits:]
                    while head:
                        chunk, head = head[:max_waits], head[max_waits:]
                        uid += 1
                        out.append(
                            {
                                "engine": ins["engine"],
                                "ins": [],
                                "is_reset_sema": False,
                                "name": f"{ins['name']}-wsplit{uid}",
                                "opcode": "Drain",
                                "outs": [],
                                "sync_info": {"on_update": [], "on_wait": chunk},
                            }
                        )
                    sy["on_wait"] = keep
                out.append(ins)
            blk["instructions"] = out
    return json.dumps(m).encode()


B, S, DIM = 4, 2048, 1024
H = DIM // 2
NCORES = 8
TOK = S // 2           # tokens (queries and keys) owned per core
SCALE = 1.0 / math.sqrt(DIM)

BF16 = mybir.dt.bfloat16
F32 = mybir.dt.float32

DT = DIM // 128        # 8  contraction tiles over model dim
CT = DIM // 128        # 8  feature tiles of Q^T/K^T
KT = S // 128          # 16 key tiles (global)
KTH = TOK // 128       # 8  key tiles per core half
NQC = TOK // 512       # 2  query chunks of 512

# test harness hooks (the grader never touches these)
TRACE = False
LAST_RESULTS = None


def _build_bass():
    nc = bass.Bass(
        trn_type="TRN2",
        target_bir_lowering=False,
        debug=False,
        num_devices=NCORES,
    )

    xth = nc.dram_tensor("xth", [DIM, TOK], BF16, kind="ExternalInput")
    wqt = nc.dram_tensor("wqt", [DIM, DIM], BF16, kind="ExternalInput")
    wkt = nc.dram_tensor("wkt", [DIM, DIM], BF16, kind="ExternalInput")
    wvt = nc.dram_tensor("wvt", [DIM, DIM], BF16, kind="ExternalInput")
    bqr = nc.dram_tensor("bqr", [128, CT], F32, kind="ExternalInput")
    bkr = nc.dram_tensor("bkr", [128, CT], F32, kind="ExternalInput")
    bvb = nc.dram_tensor("bvb", [128, DIM], F32, kind="ExternalInput")
    scv = nc.dram_tensor("scv", [128, 1], F32, kind="ExternalInput")
    outp = nc.dram_tensor("out", [TOK, DIM], BF16, kind="ExternalOutput")

    Id = mybir.ActivationFunctionType.Identity
    Exp = mybir.ActivationFunctionType.Exp
    Ln = mybir.ActivationFunctionType.Ln
    PAIRS = [[0, 1], [2, 3], [4, 5], [6, 7]]

    with tile.TileContext(nc) as tc, ExitStack() as ctx:
        const = ctx.enter_context(tc.tile_pool(name="const", bufs=1))
        persist = ctx.enter_context(tc.tile_pool(name="persist", bufs=1))
        dram = ctx.enter_context(tc.tile_pool(name="dram", bufs=1, space="DRAM"))
        ps_s = ctx.enter_context(tc.tile_pool(name="ps_s", bufs=4, space="PSUM"))

        bq_sb = const.tile([128, CT], F32)
        nc.sync.dma_start(out=bq_sb[:, :], in_=bqr[:, :])
        bk_sb = const.tile([128, CT], F32)
        nc.sync.dma_start(out=bk_sb[:, :], in_=bkr[:, :])
        sc_sb = const.tile([128, 1], F32)
        nc.sync.dma_start(out=sc_sb[:, :], in_=scv[:, :])
        ones_sb = const.tile([128, 2], F32)
        nc.vector.memset(ones_sb[:, :], 1.0)

        # Warm the PE clock gate (HAM) during the initial input-DMA wait:
        # a chain of tiny dependent matmuls gives ~4.5 us of sustained PE
        # activity so the first projection matmuls run at 2.4 GHz, not 1.2.
        with tc.psum_pool(name="ps_w", bufs=1) as ps_w:
            warm = ps_w.tile([2, 2], F32, name="warm")
            for _ in range(40):
                nc.tensor.matmul(
                    warm[:, :], ones_sb[:, :], ones_sb[:, :], start=True, stop=True
                )

        # ones stationary for the rowsum matmul (column sums, replicated
        # across partitions)
        ones_f = const.tile([128, 128], F32)
        nc.vector.memset(ones_f[:, :], 1.0)
        ones_bf = const.tile([128, 128], BF16)
        nc.vector.tensor_copy(ones_bf[:, :], ones_f[:, :])
        lnsc_sb = const.tile([128, 1], F32)
        nc.scalar.activation(lnsc_sb[:, :], sc_sb[:, :], Ln)

        # persistent products
        q_sb = [persist.tile([128, TOK], BF16, name=f"q{i}") for i in range(CT)]
        k_sb = [persist.tile([128, S], BF16, name=f"k{i}") for i in range(CT)]
        v_sb = [persist.tile([128, DIM], BF16, name=f"v{i}") for i in range(KT)]

        # DRAM bounce buffers for the pair exchange. The K gather is split
        # in two so the first feature half starts flying while the second
        # is still projecting (and before the CC-stream prelude barrier
        # would otherwise serialize one big transfer).
        k_send = [dram.tile([128, 4 * TOK], BF16, name=f"k_send{h}") for h in range(2)]
        k_recv = [
            dram.tile([2, 128, 4 * TOK], BF16, name=f"k_recv{h}") for h in range(2)
        ]
        v_send = dram.tile([128, KTH * DIM], BF16, name="v_send")
        v_recv = dram.tile([2, 128, KTH * DIM], BF16, name="v_recv")

        # XT tiles feed K, V and Q projections
        xtp = tc.alloc_tile_pool(name="xtp", bufs=1)
        x_t = [xtp.tile([128, TOK], BF16, name=f"x{d}") for d in range(DT)]

        # All input loads issue upfront on the sync-engine queue; staging
        # sends ride the scalar-engine HWDGE queue so they flow at compute
        # pace instead of FIFO-ing behind the bulk input transfers, and the
        # sync engine's blocking waits on the collective outputs never gate
        # any other issue.
        weights = tc.alloc_tile_pool(name="weights", bufs=1)
        wk_t = [weights.tile([128, DIM], BF16, name=f"wk{d}") for d in range(DT)]
        wv_t = [weights.tile([128, DIM], BF16, name=f"wv{d}") for d in range(DT)]
        wq_t = [weights.tile([128, DIM], BF16, name=f"wq{d}") for d in range(DT)]
        bv_sb = weights.tile([128, DIM], F32, name="bv_sb")
        for d in range(DT):
            nc.sync.dma_start(out=x_t[d][:, :], in_=xth[d * 128 : (d + 1) * 128, :])
            nc.sync.dma_start(out=wk_t[d][:, :], in_=wkt[d * 128 : (d + 1) * 128, :])
        for d in range(DT):
            nc.sync.dma_start(out=wv_t[d][:, :], in_=wvt[d * 128 : (d + 1) * 128, :])
        nc.sync.dma_start(out=bv_sb[:, :], in_=bvb[:, :])
        for d in range(DT):
            nc.sync.dma_start(out=wq_t[d][:, :], in_=wqt[d * 128 : (d + 1) * 128, :])

        # ---- Phase K: K^T[c, own keys] = Wk^T.T @ X^T  (+bk) ----
        kstage = tc.alloc_tile_pool(name="kstage", bufs=1)
        k_st = [kstage.tile([128, TOK], BF16, name=f"ks{c}") for c in range(CT)]
        for c in range(CT):
            for n in range(TOK // 512):
                ps = ps_s.tile([128, 512], F32, tag="ps", name="psk")
                for d in range(DT):
                    nc.tensor.matmul(
                        ps[:, :],
                        wk_t[d][:, c * 128 : (c + 1) * 128],
                        x_t[d][:, n * 512 : (n + 1) * 512],
                        start=(d == 0),
                        stop=(d == DT - 1),
                    )
                nc.scalar.activation(
                    k_st[c][:, n * 512 : (n + 1) * 512],
                    ps[:, :],
                    Id,
                    bias=bk_sb[:, c : c + 1],
                )
            nc.scalar.dma_start(
                out=k_send[c // 4][:, (c % 4) * TOK : (c % 4 + 1) * TOK],
                in_=k_st[c][:, :],
            )
            if c % 4 == 3:
                # pair-wise AllGather: K^T feature half (rank order ==
                # key-half order)
                nc.gpsimd.collective_compute(
                    "AllGather",
                    mybir.AluOpType.bypass,
                    replica_groups=PAIRS,
                    ins=[k_send[c // 4][:, :].opt()],
                    outs=[k_recv[c // 4][:, :, :].opt()],
                )

        for h in range(2):
            for g in range(2):
                for ci in range(4):
                    c = 4 * h + ci
                    nc.sync.dma_start(
                        out=k_sb[c][:, g * TOK : (g + 1) * TOK],
                        in_=k_recv[h][g, :, ci * TOK : (ci + 1) * TOK],
                    )

        # ---- Phase V: V[own keys, :] = X^T.T @ Wv^T  (+bv) ----
        vstage = tc.alloc_tile_pool(name="vstage", bufs=1)
        v_st = [vstage.tile([128, DIM], BF16, name=f"vs{i}") for i in range(KTH)]
        for i in range(KTH):
            for n in range(DIM // 512):
                ps = ps_s.tile([128, 512], F32, tag="ps", name="psv")
                for d in range(DT):
                    nc.tensor.matmul(
                        ps[:, :],
                        x_t[d][:, i * 128 : (i + 1) * 128],
                        wv_t[d][:, n * 512 : (n + 1) * 512],
                        start=(d == 0),
                        stop=(d == DT - 1),
                    )
                nc.vector.tensor_add(
                    v_st[i][:, n * 512 : (n + 1) * 512],
                    ps[:, :],
                    bv_sb[:, n * 512 : (n + 1) * 512],
                )
            nc.scalar.dma_start(
                out=v_send[:, i * DIM : (i + 1) * DIM], in_=v_st[i][:, :]
            )

        # pair-wise AllGather: V halves
        nc.gpsimd.collective_compute(
            "AllGather",
            mybir.AluOpType.bypass,
            replica_groups=PAIRS,
            ins=[v_send[:, :].opt()],
            outs=[v_recv[:, :, :].opt()],
        )
        for g in range(2):
            for i in range(KTH):
                nc.sync.dma_start(
                    out=v_sb[g * KTH + i][:, :],
                    in_=v_recv[g, :, i * DIM : (i + 1) * DIM],
                )

        # ---- Phase Q: Q^T[c, own queries] = Wq^T.T @ X^T  (+bq) ----
        for c in range(CT):
            for n in range(TOK // 512):
                ps = ps_s.tile([128, 512], F32, tag="ps", name="psq")
                for d in range(DT):
                    nc.tensor.matmul(
                        ps[:, :],
                        wq_t[d][:, c * 128 : (c + 1) * 128],
                        x_t[d][:, n * 512 : (n + 1) * 512],
                        start=(d == 0),
                        stop=(d == DT - 1),
                    )
                nc.scalar.activation(
                    q_sb[c][:, n * 512 : (n + 1) * 512],
                    ps[:, :],
                    Id,
                    bias=bq_sb[:, c : c + 1],
                )
        vstage.release()
        kstage.release()
        weights.release()
        xtp.release()

        # ---- Phase 2: attention ----
        # P^T tiles for both query chunks stay live so chunk 1's score
        # matmuls overlap chunk 0's DVE combine.
        with (
            tc.tile_pool(name="pP", bufs=1) as pP,
            tc.tile_pool(name="ps_r", bufs=2, space="PSUM") as ps_r,
            tc.tile_pool(name="ps_u", bufs=2, space="PSUM") as ps_u,
            tc.tile_pool(name="small", bufs=8) as small,
            tc.tile_pool(name="tmp2", bufs=2) as tmp2,
            tc.tile_pool(name="ostage", bufs=3) as ostage,
        ):
            p_sb = [
                [
                    [pP.tile([128, 512], BF16, name=f"p{qc}_{j}_{k}") for k in range(KT)]
                    for j in range(2)
                ]
                for qc in range(NQC)
            ]
            bcs = [[None, None] for _ in range(NQC)]

            def emit_scores(qc):
                # scores S^T[k, q] = K_j^T.T @ Q_j^T; P = exp(s*S^T);
                # r = column sums via ones-matmul, trailed by 2 tiles so the
                # Scalar exp latency stays off the PE critical path.
                for j in range(2):
                    r_ps = ps_r.tile([128, 512], F32, tag="r", name=f"r{qc}{j}")
                    for k in range(KT):
                        ps = ps_s.tile([128, 512], F32, tag="ps", name="pss")
                        for ci in range(4):
                            c = 4 * j + ci
                            nc.tensor.matmul(
                                ps[:, :],
                                k_sb[c][:, k * 128 : (k + 1) * 128],
                                q_sb[c][:, qc * 512 : (qc + 1) * 512],
                                start=(ci == 0),
                                stop=(ci == 3),
                            )
                        nc.scalar.activation(
                            p_sb[qc][j][k][:, :], ps[:, :], Exp, scale=SCALE
                        )
                        if k >= 2:
                            nc.tensor.matmul(
                                r_ps[:, :],
                                ones_bf[:, :],
                                p_sb[qc][j][k - 2][:, :],
                                start=(k == 2),
                                stop=False,
                            )
                    for k in (KT - 2, KT - 1):
                        nc.tensor.matmul(
                            r_ps[:, :],
                            ones_bf[:, :],
                            p_sb[qc][j][k][:, :],
                            start=False,
                            stop=(k == KT - 1),
                        )
                    # bc_j = exp(-ln r_j) = 1/r_j on the Scalar engine
                    # (j=1 folds the input scalar in via a +ln(scalar) bias)
                    lnr = tmp2.tile([128, 512], F32, tag="lnr", name="lnr")
                    nc.scalar.activation(lnr[:, :], r_ps[:, :], Ln)
                    bc = small.tile([128, 512], BF16, tag=f"bc{qc}{j}", name=f"bc{qc}{j}")
                    if j == 0:
                        nc.scalar.activation(bc[:, :], lnr[:, :], Exp, scale=-1.0)
                    else:
                        nc.scalar.activation(
                            bc[:, :], lnr[:, :], Exp, scale=-1.0, bias=lnsc_sb[:, :]
                        )
                    bcs[qc][j] = bc

            def emit_combine(qc):
                # A^T[k] = P1[k]*bc1 - P2[k]*bc2s  (in place into p_sb[qc][1])
                for k in range(KT):
                    t2 = tmp2.tile([128, 512], BF16, tag="t2", name="t2")
                    nc.vector.tensor_mul(t2[:, :], p_sb[qc][0][k][:, :], bcs[qc][0][:, :])
                    nc.vector.tensor_mul(
                        p_sb[qc][1][k][:, :], p_sb[qc][1][k][:, :], bcs[qc][1][:, :]
                    )
                    nc.vector.tensor_sub(
                        p_sb[qc][1][k][:, :], t2[:, :], p_sb[qc][1][k][:, :]
                    )

            def emit_attnv(qc):
                # out rows = A^T.T @ V
                for t in range(4):
                    row = qc * 512 + t * 128
                    for n in range(DIM // 512):
                        lo, hi = n * 512, (n + 1) * 512
                        u = ps_u.tile([128, 512], F32, tag="u", name="u")
                        for k in range(KT):
                            nc.tensor.matmul(
                                u[:, :],
                                p_sb[qc][1][k][:, t * 128 : (t + 1) * 128],
                                v_sb[k][:, lo:hi],
                                start=(k == 0),
                                stop=(k == KT - 1),
                            )
                        o = ostage.tile([128, 512], BF16, tag="o", name="o")
                        if n == 0:
                            nc.scalar.copy(o[:, :], u[:, :])
                        else:
                            nc.vector.tensor_copy(o[:, :], u[:, :])
                        nc.sync.dma_start(out=outp[row : row + 128, lo:hi], in_=o[:, :])

            emit_scores(0)
            emit_combine(0)
            emit_scores(1)
            emit_combine(1)
            emit_attnv(0)
            emit_attnv(1)

    return nc


_NC_CACHE = None


def _get_nc():
    global _NC_CACHE
    if _NC_CACHE is None:
        nc = _build_bass()
        fixed = _split_waits(bass.Bass.to_json_bytes(nc))
        nc.to_json_bytes = lambda: fixed
        _NC_CACHE = nc
    return _NC_CACHE


def kernel(hidden_states, W_q, b_q, W_k, b_k, W_v, b_v, scalar):
    global LAST_RESULTS
    bf16 = ml_dtypes.bfloat16
    X = np.asarray(hidden_states, np.float32)
    wqt = np.ascontiguousarray(np.asarray(W_q, np.float32).T).astype(bf16)
    wkt = np.ascontiguousarray(np.asarray(W_k, np.float32).T).astype(bf16)
    wvt = np.ascontiguousarray(np.asarray(W_v, np.float32).T).astype(bf16)
    bqr = np.ascontiguousarray(np.asarray(b_q, np.float32).reshape(CT, 128).T)
    bkr = np.ascontiguousarray(np.asarray(b_k, np.float32).reshape(CT, 128).T)
    bvb = np.ascontiguousarray(
        np.broadcast_to(np.asarray(b_v, np.float32), (128, DIM))
    )
    scv = np.full((128, 1), np.asarray(scalar, np.float32).reshape(-1)[0], np.float32)

    in_maps = []
    for core in range(NCORES):
        b, h = core // 2, core % 2
        xt_b = np.ascontiguousarray(X[b, h * TOK : (h + 1) * TOK, :].T).astype(bf16)
        in_maps.append(
            {
                "xth": xt_b,
                "wqt": wqt,
                "wkt": wkt,
                "wvt": wvt,
                "bqr": bqr,
                "bkr": bkr,
                "bvb": bvb,
                "scv": scv,
            }
        )

    nc = _get_nc()
    res = run_bass_kernel_spmd(
        nc,
        in_maps,
        list(range(NCORES)),
        trace=TRACE,
    )
    LAST_RESULTS = res

    out = np.empty((B, S, DIM), np.float32)
    for core in range(NCORES):
        b, h = core // 2, core % 2
        out[b, h * TOK : (h + 1) * TOK, :] = res.results[core]["out"].astype(np.float32)
    return out


if __name__ == "__main__":
    import reference

    inputs = {k: np.asarray(v) for k, v in reference.setup_inputs().items()}
    got = kernel(**inputs)
    print("kernel output", got.shape, got.dtype)


# revision 10
# speedup vs baseline: 1.3462x; 1.0313x over previous
"""Trainium2 Bass kernel for nn_DiffAttn (differential attention).

Reference computation (per batch b):
    Q = X @ Wq.T + bq ; K = X @ Wk.T + bk ; V = X @ Wv.T + bv
    Q1,Q2 / K1,K2 = halves of feature dim
    A_j = (Q_j @ K_j.T) / sqrt(DIM)
    out = softmax(A1) @ V - scalar * softmax(A2) @ V

Sharding: 8 cores = 4 batches x 2 token-halves. Each core projects
Q/K/V only for its OWN 1024 tokens; the K^T/V halves are exchanged
inside each batch pair with two pair-wise AllGather collectives
(HBM bounce buffers), so no projection work is duplicated. The gather
output is rank-ordered == key-half-ordered, so every core addresses
K/V tiles by global key index and the program is SPMD-uniform; the
only per-core data is the X^T token slab (and the host assembles the
output slabs).

All matmuls run in bf16 (fp32r moving operands measure ~2x slower per
column on TRN2 hardware): projections, scores, rowsums (ones-matmul),
and attn@V. P = exp(scores) is stored bf16; attention weights are
normalized BEFORE the V matmul (A = P1/r1 - scalar*P2/r2, with 1/r =
exp(-ln r) on the Scalar engine) so a single attn@V GEMM suffices.
The two query chunks are software-pipelined: the DVE combine of chunk
0 overlaps the score matmuls of chunk 1, and rowsum matmuls trail the
score chains by two tiles so the Scalar-engine exp latency stays off
the PE critical path. Output is written bf16 and widened on the host.
"""

import json
import math
from contextlib import ExitStack

import numpy as np
import ml_dtypes

import concourse.bass as bass
import concourse.tile as tile
from concourse import mybir
from concourse.bass_utils import run_bass_kernel_spmd


def _split_waits(raw: bytes, max_waits: int = 1) -> bytes:
    """walrus's CoreV3 codegen rejects instructions carrying more than one
    sync wait ("Too many sync wait commands"); Tile's kernel-tail drain
    aggregates one wait per live processor. Hoist excess waits onto chained
    same-engine Drain instructions inserted immediately before the offender."""
    m = json.loads(raw)
    uid = 0
    for fn in m["functions"]:
        for blk in fn["blocks"]:
            out = []
            for ins in blk["instructions"]:
                sy = ins.get("sync_info") or {}
                waits = sy.get("on_wait") or []
                if len(waits) > max_waits:
                    head, keep = waits[:-max_waits], waits[-max_waits:]
                    while head:
                        chunk, head = head[:max_waits], head[max_waits:]
                        uid += 1
                        out.append(
                            {
                                "engine": ins["engine"],
                                "ins": [],
                                "is_reset_sema": False,
                                "name": f"{ins['name']}-wsplit{uid}",
                                "opcode": "Drain",
                                "outs": [],
                                "sync_info": {"on_update": [], "on_wait": chunk},
                            }
                        )
                    sy["on_wait"] = keep
                out.append(ins)
            blk["instructions"] = out
    return json.dumps(m).encode()


B, S, DIM = 4, 2048, 1024
H = DIM // 2
NCORES = 8
TOK = S // 2           # tokens (queries and keys) owned per core
SCALE = 1.0 / math.sqrt(DIM)

BF16 = mybir.dt.bfloat16
F32 = mybir.dt.float32

DT = DIM // 128        # 8  contraction tiles over model dim
CT = DIM // 128        # 8  feature tiles of Q^T/K^T
KT = S // 128          # 16 key tiles (global)
KTH = TOK // 128       # 8  key tiles per core half
NQC = TOK // 512       # 2  query chunks of 512

# test harness hooks (the grader never touches these)
TRACE = False
LAST_RESULTS = None


def _build_bass():
    nc = bass.Bass(
        trn_type="TRN2",
        target_bir_lowering=False,
        debug=False,
        num_devices=NCORES,
    )

    xth = nc.dram_tensor("xth", [DIM, TOK], BF16, kind="ExternalInput")
    wqt = nc.dram_tensor("wqt", [DIM, DIM], BF16, kind="ExternalInput")
    wkt = nc.dram_tensor("wkt", [DIM, DIM], BF16, kind="ExternalInput")
    wvt = nc.dram_tensor("wvt", [DIM, DIM], BF16, kind="ExternalInput")
    bqr = nc.dram_tensor("bqr", [128, CT], F32, kind="ExternalInput")
    bkr = nc.dram_tensor("bkr", [128, CT], F32, kind="ExternalInput")
    bvb = nc.dram_tensor("bvb", [128, DIM], F32, kind="ExternalInput")
    scv = nc.dram_tensor("scv", [128, 1], F32, kind="ExternalInput")
    outp = nc.dram_tensor("out", [TOK, DIM], BF16, kind="ExternalOutput")

    Id = mybir.ActivationFunctionType.Identity
    Exp = mybir.ActivationFunctionType.Exp
    Ln = mybir.ActivationFunctionType.Ln
    PAIRS = [[0, 1], [2, 3], [4, 5], [6, 7]]

    with tile.TileContext(nc) as tc, ExitStack() as ctx:
        const = ctx.enter_context(tc.tile_pool(name="const", bufs=1))
        persist = ctx.enter_context(tc.tile_pool(name="persist", bufs=1))
        dram = ctx.enter_context(tc.tile_pool(name="dram", bufs=1, space="DRAM"))
        ps_s = ctx.enter_context(tc.tile_pool(name="ps_s", bufs=6, space="PSUM"))

        bq_sb = const.tile([128, CT], F32)
        nc.sync.dma_start(out=bq_sb[:, :], in_=bqr[:, :])
        bk_sb = const.tile([128, CT], F32)
        nc.sync.dma_start(out=bk_sb[:, :], in_=bkr[:, :])
        sc_sb = const.tile([128, 1], F32)
        nc.sync.dma_start(out=sc_sb[:, :], in_=scv[:, :])
        ones_sb = const.tile([128, 2], F32)
        nc.vector.memset(ones_sb[:, :], 1.0)

        # Warm the PE clock gate (HAM) during the initial input-DMA wait:
        # a chain of tiny dependent matmuls gives ~4.5 us of sustained PE
        # activity so the first projection matmuls run at 2.4 GHz, not 1.2.
        with tc.psum_pool(name="ps_w", bufs=1) as ps_w:
            warm = ps_w.tile([2, 2], F32, name="warm")
            for _ in range(40):
                nc.tensor.matmul(
                    warm[:, :], ones_sb[:, :], ones_sb[:, :], start=True, stop=True
                )

        # ones stationary for the rowsum matmul (column sums, replicated
        # across partitions)
        ones_f = const.tile([128, 128], F32)
        nc.vector.memset(ones_f[:, :], 1.0)
        ones_bf = const.tile([128, 128], BF16)
        nc.vector.tensor_copy(ones_bf[:, :], ones_f[:, :])
        lnsc_sb = const.tile([128, 1], F32)
        nc.scalar.activation(lnsc_sb[:, :], sc_sb[:, :], Ln)

        # persistent products
        q_sb = [persist.tile([128, TOK], BF16, name=f"q{i}") for i in range(CT)]
        k_sb = [persist.tile([128, S], BF16, name=f"k{i}") for i in range(CT)]
        v_sb = [persist.tile([128, DIM], BF16, name=f"v{i}") for i in range(KT)]

        # DRAM bounce buffers for the pair exchange. The K gather is split
        # in two so the first feature half starts flying while the second
        # is still projecting (and before the CC-stream prelude barrier
        # would otherwise serialize one big transfer).
        k_send = [dram.tile([128, 4 * TOK], BF16, name=f"k_send{h}") for h in range(2)]
        k_recv = [
            dram.tile([2, 128, 4 * TOK], BF16, name=f"k_recv{h}") for h in range(2)
        ]
        v_send = dram.tile([128, KTH * DIM], BF16, name="v_send")
        v_recv = dram.tile([2, 128, KTH * DIM], BF16, name="v_recv")

        # XT tiles feed K, V and Q projections
        xtp = tc.alloc_tile_pool(name="xtp", bufs=1)
        x_t = [xtp.tile([128, TOK], BF16, name=f"x{d}") for d in range(DT)]

        # All input loads issue upfront on the sync-engine queue; staging
        # sends ride the scalar-engine HWDGE queue so they flow at compute
        # pace instead of FIFO-ing behind the bulk input transfers, and the
        # sync engine's blocking waits on the collective outputs never gate
        # any other issue.
        weights = tc.alloc_tile_pool(name="weights", bufs=1)
        wk_t = [weights.tile([128, DIM], BF16, name=f"wk{d}") for d in range(DT)]
        wv_t = [weights.tile([128, DIM], BF16, name=f"wv{d}") for d in range(DT)]
        wq_t = [weights.tile([128, DIM], BF16, name=f"wq{d}") for d in range(DT)]
        bv_sb = weights.tile([128, DIM], F32, name="bv_sb")
        for d in range(DT):
            nc.sync.dma_start(out=x_t[d][:, :], in_=xth[d * 128 : (d + 1) * 128, :])
            nc.sync.dma_start(out=wk_t[d][:, :], in_=wkt[d * 128 : (d + 1) * 128, :])
        for d in range(DT):
            nc.sync.dma_start(out=wv_t[d][:, :], in_=wvt[d * 128 : (d + 1) * 128, :])
        nc.sync.dma_start(out=bv_sb[:, :], in_=bvb[:, :])
        for d in range(DT):
            nc.sync.dma_start(out=wq_t[d][:, :], in_=wqt[d * 128 : (d + 1) * 128, :])

        # ---- Phase K: K^T[c, own keys] = Wk^T.T @ X^T  (+bk) ----
        kstage = tc.alloc_tile_pool(name="kstage", bufs=1)
        k_st = [kstage.tile([128, TOK], BF16, name=f"ks{c}") for c in range(CT)]
        for c in range(CT):
            for n in range(TOK // 512):
                ps = ps_s.tile([128, 512], F32, tag="ps", name="psk")
                for d in range(DT):
                    nc.tensor.matmul(
                        ps[:, :],
                        wk_t[d][:, c * 128 : (c + 1) * 128],
                        x_t[d][:, n * 512 : (n + 1) * 512],
                        start=(d == 0),
                        stop=(d == DT - 1),
                    )
                nc.scalar.activation(
                    k_st[c][:, n * 512 : (n + 1) * 512],
                    ps[:, :],
                    Id,
                    bias=bk_sb[:, c : c + 1],
                )
            nc.scalar.dma_start(
                out=k_send[c // 4][:, (c % 4) * TOK : (c % 4 + 1) * TOK],
                in_=k_st[c][:, :],
            )
            if c % 4 == 3:
                # pair-wise AllGather: K^T feature half (rank order ==
                # key-half order)
                nc.gpsimd.collective_compute(
                    "AllGather",
                    mybir.AluOpType.bypass,
                    replica_groups=PAIRS,
                    ins=[k_send[c // 4][:, :].opt()],
                    outs=[k_recv[c // 4][:, :, :].opt()],
                )

        for h in range(2):
            for g in range(2):
                for ci in range(4):
                    c = 4 * h + ci
                    nc.sync.dma_start(
                        out=k_sb[c][:, g * TOK : (g + 1) * TOK],
                        in_=k_recv[h][g, :, ci * TOK : (ci + 1) * TOK],
                    )

        # ---- Phase V: V[own keys, :] = X^T.T @ Wv^T  (+bv) ----
        vstage = tc.alloc_tile_pool(name="vstage", bufs=1)
        v_st = [vstage.tile([128, DIM], BF16, name=f"vs{i}") for i in range(KTH)]
        for i in range(KTH):
            for n in range(DIM // 512):
                ps = ps_s.tile([128, 512], F32, tag="ps", name="psv")
                for d in range(DT):
                    nc.tensor.matmul(
                        ps[:, :],
                        x_t[d][:, i * 128 : (i + 1) * 128],
                        wv_t[d][:, n * 512 : (n + 1) * 512],
                        start=(d == 0),
                        stop=(d == DT - 1),
                    )
                nc.vector.tensor_add(
                    v_st[i][:, n * 512 : (n + 1) * 512],
                    ps[:, :],
                    bv_sb[:, n * 512 : (n + 1) * 512],
                )
            nc.scalar.dma_start(
                out=v_send[:, i * DIM : (i + 1) * DIM], in_=v_st[i][:, :]
            )

        # pair-wise AllGather: V halves
        nc.gpsimd.collective_compute(
            "AllGather",
            mybir.AluOpType.bypass,
            replica_groups=PAIRS,
            ins=[v_send[:, :].opt()],
            outs=[v_recv[:, :, :].opt()],
        )
        for g in range(2):
            for i in range(KTH):
                nc.sync.dma_start(
                    out=v_sb[g * KTH + i][:, :],
                    in_=v_recv[g, :, i * DIM : (i + 1) * DIM],
                )

        # ---- Phase Q: Q^T[c, own queries] = Wq^T.T @ X^T  (+bq) ----
        for c in range(CT):
            for n in range(TOK // 512):
                ps = ps_s.tile([128, 512], F32, tag="ps", name="psq")
                for d in range(DT):
                    nc.tensor.matmul(
                        ps[:, :],
                        wq_t[d][:, c * 128 : (c + 1) * 128],
                        x_t[d][:, n * 512 : (n + 1) * 512],
                        start=(d == 0),
                        stop=(d == DT - 1),
                    )
                nc.scalar.activation(
                    q_sb[c][:, n * 512 : (n + 1) * 512],
                    ps[:, :],
                    Id,
                    bias=bq_sb[:, c : c + 1],
                )
        vstage.release()
        kstage.release()
        weights.release()
        xtp.release()

        # ---- Phase 2: attention ----
        # P^T tiles for both query chunks stay live so chunk 1's score
        # matmuls overlap chunk 0's DVE combine.
        with (
            tc.tile_pool(name="pP", bufs=1) as pP,
            tc.tile_pool(name="ps_r", bufs=2, space="PSUM") as ps_r,
            tc.tile_pool(name="small", bufs=8) as small,
            tc.tile_pool(name="tmp2", bufs=2) as tmp2,
            tc.tile_pool(name="ostage", bufs=3) as ostage,
        ):
            p_sb = [
                [
                    [pP.tile([128, 512], BF16, name=f"p{qc}_{j}_{k}") for k in range(KT)]
                    for j in range(2)
                ]
                for qc in range(NQC)
            ]
            bcs = [[None, None] for _ in range(NQC)]

            def emit_scores(qc):
                # scores S^T[k, q] = K_j^T.T @ Q_j^T; P = exp(s*S^T);
                # r = column sums via ones-matmul, trailed by 2 tiles so the
                # Scalar exp latency stays off the PE critical path.
                for j in range(2):
                    r_ps = ps_r.tile([128, 512], F32, tag="r", name=f"r{qc}{j}")
                    for k in range(KT):
                        ps = ps_s.tile([128, 512], F32, tag="ps", name="pss")
                        for ci in range(4):
                            c = 4 * j + ci
                            nc.tensor.matmul(
                                ps[:, :],
                                k_sb[c][:, k * 128 : (k + 1) * 128],
                                q_sb[c][:, qc * 512 : (qc + 1) * 512],
                                start=(ci == 0),
                                stop=(ci == 3),
                            )
                        nc.scalar.activation(
                            p_sb[qc][j][k][:, :], ps[:, :], Exp, scale=SCALE
                        )
                        if k >= 2:
                            nc.tensor.matmul(
                                r_ps[:, :],
                                ones_bf[:, :],
                                p_sb[qc][j][k - 2][:, :],
                                start=(k == 2),
                                stop=False,
                            )
                    for k in (KT - 2, KT - 1):
                        nc.tensor.matmul(
                            r_ps[:, :],
                            ones_bf[:, :],
                            p_sb[qc][j][k][:, :],
                            start=False,
                            stop=(k == KT - 1),
                        )
                    # bc_j = exp(-ln r_j) = 1/r_j on the Scalar engine
                    # (j=1 folds the input scalar in via a +ln(scalar) bias)
                    lnr = tmp2.tile([128, 512], F32, tag="lnr", name="lnr")
                    nc.scalar.activation(lnr[:, :], r_ps[:, :], Ln)
                    bc = small.tile([128, 512], BF16, tag=f"bc{qc}{j}", name=f"bc{qc}{j}")
                    if j == 0:
                        nc.scalar.activation(bc[:, :], lnr[:, :], Exp, scale=-1.0)
                    else:
                        nc.scalar.activation(
                            bc[:, :], lnr[:, :], Exp, scale=-1.0, bias=lnsc_sb[:, :]
                        )
                    bcs[qc][j] = bc

            def emit_combine(qc):
                # A^T[k] = P1[k]*bc1 - P2[k]*bc2s  (in place into p_sb[qc][1])
                for k in range(KT):
                    t2 = tmp2.tile([128, 512], BF16, tag="t2", name="t2")
                    nc.vector.tensor_mul(t2[:, :], p_sb[qc][0][k][:, :], bcs[qc][0][:, :])
                    nc.vector.tensor_mul(
                        p_sb[qc][1][k][:, :], p_sb[qc][1][k][:, :], bcs[qc][1][:, :]
                    )
                    nc.vector.tensor_sub(
                        p_sb[qc][1][k][:, :], t2[:, :], p_sb[qc][1][k][:, :]
                    )

            def emit_attnv(qc):
                # out rows = A^T.T @ V
                for t in range(4):
                    row = qc * 512 + t * 128
                    for n in range(DIM // 512):
                        lo, hi = n * 512, (n + 1) * 512
                        u = ps_s.tile([128, 512], F32, tag="ps", name="u")
                        for k in range(KT):
                            nc.tensor.matmul(
                                u[:, :],
                                p_sb[qc][1][k][:, t * 128 : (t + 1) * 128],
                                v_sb[k][:, lo:hi],
                                start=(k == 0),
                                stop=(k == KT - 1),
                            )
                        o = ostage.tile([128, 512], BF16, tag="o", name="o")
                        nc.scalar.copy(o[:, :], u[:, :])
                        nc.sync.dma_start(out=outp[row : row + 128, lo:hi], in_=o[:, :])

            emit_scores(0)
            emit_combine(0)
            emit_scores(1)
            emit_combine(1)
            emit_attnv(0)
            emit_attnv(1)

    return nc


_NC_CACHE = None


def _get_nc():
    global _NC_CACHE
    if _NC_CACHE is None:
        nc = _build_bass()
        fixed = _split_waits(bass.Bass.to_json_bytes(nc))
        nc.to_json_bytes = lambda: fixed
        _NC_CACHE = nc
    return _NC_CACHE


def kernel(hidden_states, W_q, b_q, W_k, b_k, W_v, b_v, scalar):
    global LAST_RESULTS
    bf16 = ml_dtypes.bfloat16
    X = np.asarray(hidden_states, np.float32)
    wqt = np.ascontiguousarray(np.asarray(W_q, np.float32).T).astype(bf16)
    wkt = np.ascontiguousarray(np.asarray(W_k, np.float32).T).astype(bf16)
    wvt = np.ascontiguousarray(np.asarray(W_v, np.float32).T).astype(bf16)
    bqr = np.ascontiguousarray(np.asarray(b_q, np.float32).reshape(CT, 128).T)
    bkr = np.ascontiguousarray(np.asarray(b_k, np.float32).reshape(CT, 128).T)
    bvb = np.ascontiguousarray(
        np.broadcast_to(np.asarray(b_v, np.float32), (128, DIM))
    )
    scv = np.full((128, 1), np.asarray(scalar, np.float32).reshape(-1)[0], np.float32)

    in_maps = []
    for core in range(NCORES):
        b, h = core // 2, core % 2
        xt_b = np.ascontiguousarray(X[b, h * TOK : (h + 1) * TOK, :].T).astype(bf16)
        in_maps.append(
            {
                "xth": xt_b,
                "wqt": wqt,
                "wkt": wkt,
                "wvt": wvt,
                "bqr": bqr,
                "bkr": bkr,
                "bvb": bvb,
                "scv": scv,
            }
        )

    nc = _get_nc()
    res = run_bass_kernel_spmd(
        nc,
        in_maps,
        list(range(NCORES)),
        trace=TRACE,
    )
    LAST_RESULTS = res

    out = np.empty((B, S, DIM), np.float32)
    for core in range(NCORES):
        b, h = core // 2, core % 2
        out[b, h * TOK : (h + 1) * TOK, :] = res.results[core]["out"].astype(np.float32)
    return out


if __name__ == "__main__":
    import reference

    inputs = {k: np.asarray(v) for k, v in reference.setup_inputs().items()}
    got = kernel(**inputs)
    print("kernel output", got.shape, got.dtype)
